# revision 31
# baseline (speedup 1.0000x reference)
"""Trainium2 Bass kernel for windowed (block-diagonal) multi-head video attention.

Problem: x:[2,8192,1024] -> qkv proj -> 3D-window (2,8,8) attention over a
(8,32,32) token grid, 16 heads x 64 dim -> out proj -> [2,8192,1024].

Sharding: 8 cores, data-parallel over (batch, t-window-group): core (b, it)
owns the 2048-token slab with t in {2it, 2it+1} = 16 independent 128-token
windows.

v6 (bf16, merged pipeline): all matmuls in bf16 (1 cyc/row at any ap-size vs
f32r's 4x penalty below ap 256).  x is pre-transposed AND window-permuted on
the host, so the kernel starts from x^T [c, tok] in HBM: no gather DMAs, no
PE transposes.  The softmax scale is folded into the Q weights on the host.
Groups of 4 windows (512 tokens) are processed in a software pipeline where
iteration t emits group t's QKV projection matmuls interleaved into group
t-1's attention stream, keeping the tensor engine dense (no pstate drops)
and hiding the ACT-bound softmax-reciprocal chain (ln + exp(-x) on the psum
denominator row produced by a ones-column in V, K=1 bf16 matmul partition
broadcast, DVE multiply).  qkT / v65 are double-buffered per group; weights
are loaded block-per-tile over two fast DMA rings in consumption order so
each QK chunk depends only on its own block.  Weights/x bf16, psum f32,
output bf16 (converted to f32 on host).  Measured: 355.6us on HW
(vs 567.8us f32r baseline), rel err 4.9e-3.
"""

import sys

for _p in ("/opt/trn_rl_repo",):
    if _p not in sys.path:
        sys.path.insert(0, _p)

import numpy as np

B, T, H, W = 2, 8, 32, 32
C, NH, HD = 1024, 16, 64
WT, WH, WW = 2, 8, 8
N = T * H * W              # 8192 tokens
SCALE = HD ** -0.5
NCORES = 8
SLAB = N // (T // WT)      # 2048 tokens per (b, it) slab
NWIN = (H // WH) * (W // WW)   # 16 windows per slab
M = WT * WH * WW           # 128 tokens per window
KC = C // 128              # 8 contraction chunks
GW = 4                     # windows per group
TOKG = M * GW              # 512 tokens per group
NGRP = NWIN // GW          # 4 groups

_BUILD_CACHE = {}


def _split_drain_waits(nc, mybir, cap=1, event_cap=2):
    """This walrus build accepts only one sem wait per TPB instruction
    (Tile's scheduler attaches up to 3).  Move the excess onto
    InstEventSemaphore carriers (which hold 2) inserted right before the
    over-subscribed instruction on the same engine — the engine blocks on the
    carriers first, so semantics are unchanged."""
    for f in nc.m.functions:
        for bb in f.blocks:
            i = 0
            while i < len(bb.instructions):
                ins = bb.instructions[i]
                si = ins.sync_info
                my_cap = (
                    event_cap
                    if type(ins).__name__ == "InstEventSemaphore"
                    else cap
                )
                if si is not None and si.on_wait and len(si.on_wait) > my_cap:
                    waits = list(si.on_wait)
                    si.on_wait = waits[:my_cap]
                    extra = waits[my_cap:]
                    carriers = []
                    while extra:
                        chunk, extra = extra[:event_cap], extra[event_cap:]
                        ev = mybir.InstEventSemaphore(
                            name=f"I-{nc.next_id()}-waitsplit", ins=[], outs=[]
                        )
                        ev.engine = ins.engine
                        ev.sync_info = mybir.SyncInfo(
                            on_wait=list(chunk), on_update=[]
                        )
                        nc.register_instruction(ev)
                        carriers.append(ev)
                    bb.instructions[i:i] = carriers
                    i += len(carriers)
                i += 1


def _build(has_qkvb, has_projb):
    import concourse.bass as bass
    import concourse.tile as tile
    from concourse import mybir
    f32 = mybir.dt.float32
    f32r = mybir.dt.float32r
    bf = mybir.dt.bfloat16
    Exp = mybir.ActivationFunctionType.Exp
    Ln = mybir.ActivationFunctionType.Ln

    nc = bass.Bass("TRN2", target_bir_lowering=False, debug=False)
    # x^T, window-permuted on host: [p, kc, tok] with tok in (group, win, m)
    # order; c = kc*128 + p
    xT_d = nc.dram_tensor("xT", [128, KC, SLAB], bf, kind="ExternalInput")
    # qkv weights pre-transposed + SCALE folded into Q, blocked by 128-out-col
    # chunks so each DMA is [128, KC*128] with 2KB/partition lines
    wq_d = nc.dram_tensor("wqkvT", [3 * KC, 128, KC, 128], bf,
                          kind="ExternalInput")
    wp_d = nc.dram_tensor("projT", [KC, 128, KC, 128], bf,
                          kind="ExternalInput")
    if has_qkvb:
        qkvb = nc.dram_tensor("qkvb", [1, 3 * C], bf, kind="ExternalInput")
    if has_projb:
        projb = nc.dram_tensor("projb", [1, C], bf, kind="ExternalInput")
    out = nc.dram_tensor("out", [SLAB, C], bf, kind="ExternalOutput")
    # out rows are window-major: row = 512*g + 128*w + m
    out_v = out.ap().rearrange("(g w m) c -> g w m c", g=NGRP, w=GW)

    with tile.TileContext(nc) as tc:
        with (
            tc.tile_pool(name="const", bufs=1) as const_pool,
            tc.tile_pool(name="wq", bufs=1) as wq_pool,
            tc.tile_pool(name="wp", bufs=1) as wp_pool,
            tc.tile_pool(name="xs", bufs=1) as xs_pool,
            tc.tile_pool(name="qkt", bufs=2) as qkt_pool,
            tc.tile_pool(name="v65", bufs=2) as v_pool,
            tc.tile_pool(name="ow", bufs=1) as ow_pool,
            tc.tile_pool(name="E", bufs=2) as e_pool,
            tc.tile_pool(name="rL", bufs=1) as rL_pool,
            tc.tile_pool(name="rr", bufs=2) as r_pool,
            tc.tile_pool(name="RR", bufs=2) as R_pool,
            tc.tile_pool(name="o", bufs=2) as o_pool,
            tc.tile_pool(name="psA", bufs=2, space="PSUM") as psA,
            tc.tile_pool(name="psVA", bufs=1, space="PSUM") as psVAp,
            tc.tile_pool(name="psVB", bufs=1, space="PSUM") as psVBp,
            tc.tile_pool(name="psS", bufs=1, space="PSUM") as psSp,
        ):
            # ---- constants ----
            ones_colf = const_pool.tile([128, NWIN * GW], f32)
            nc.vector.memset(ones_colf[:], 1.0)
            ones_col = const_pool.tile([128, GW * NH], bf)
            with nc.allow_low_precision(reason="bf16 const"):
                nc.scalar.copy(ones_col[:], ones_colf[:, 0 : GW * NH])
            ones64f = const_pool.tile([1, 64], f32)
            nc.vector.memset(ones64f[:], 1.0)
            ones64r = const_pool.tile([1, 64], f32r)
            with nc.allow_low_precision(reason="f32r const"):
                nc.scalar.copy(ones64r[:], ones64f[:])
            if has_qkvb or has_projb:
                onestf = const_pool.tile([1, TOKG], f32)
                nc.vector.memset(onestf[:], 1.0)
                onest = const_pool.tile([1, TOKG], bf)
                with nc.allow_low_precision(reason="bf16 const"):
                    nc.scalar.copy(onest[:], onestf[:])
            if has_qkvb:
                qkvb_sb = const_pool.tile([1, 3 * C], bf)
                nc.sync.dma_start(qkvb_sb[:], qkvb.ap())
            if has_projb:
                projb_sb = const_pool.tile([1, C], bf)
                nc.sync.dma_start(projb_sb[:], projb.ap())

            # ---- resident tensors ----
            xT = xs_pool.tile([128, KC, SLAB], bf)
            # block-major weights, one tile per Q/K block so each QK chunk
            # depends only on its own block's DMA; V blocks share one tile
            wqk_t = []
            for _b in range(16):
                wqk_blk = wq_pool.tile([128, KC, 128], bf, tag=f"wqk{_b}")
                wqk_t.append(wqk_blk)
            wqv_sb = wq_pool.tile([128, KC, KC, 128], bf)
            wp_sb = wp_pool.tile([128, KC, KC, 128], bf)
            owT = ow_pool.tile([128, KC, TOKG], bf)
            # attention psum: two ping-pong half-window tiles (2 banks each)
            psVA = psVAp.tile([128, 2, 512], f32)
            psVB = psVBp.tile([128, 2, 512], f32)

            # DMA schedule: two fast rings (SP, ACT) carry x group-0 and the
            # qkv weight blocks in consumption order.  Only the immediately
            # needed issues go up front: when the ring backs up, the issuing
            # ENGINE blocks on dma_start, and the ACT/SP queues must stay
    
            # responsive (ACT runs the QK psum evictions that recycle psA).
            # Everything else is dribbled into the t=0 instruction stream.
            for k in range(0, KC, 2):
                nc.sync.dma_start(xT[:, k, 0:TOKG], xT_d.ap()[:, k, 0:TOKG])
            for k in range(1, KC, 2):
                nc.scalar.dma_start(xT[:, k, 0:TOKG], xT_d.ap()[:, k, 0:TOKG])
            for blk in range(4):
                (nc.sync if blk % 2 == 0 else nc.scalar).dma_start(
                    wqk_t[blk][:], wq_d.ap()[blk]
                )
            for k in range(KC):
                nc.gpsimd.dma_start(
                    xT[:, k, 2 * TOKG : 3 * TOKG],
                    xT_d.ap()[:, k, 2 * TOKG : 3 * TOKG],
                )

            late_dmas = []
            for blk in range(4, 16):
                late_dmas.append((blk % 2, lambda blk=blk: (
                    nc.sync if blk % 2 == 0 else nc.scalar).dma_start(
                        wqk_t[blk][:], wq_d.ap()[blk])))
            for blk in range(16, 24):
                late_dmas.append((blk % 2, lambda blk=blk: (
                    nc.sync if blk % 2 == 0 else nc.scalar).dma_start(
                        wqv_sb[:, blk - 16], wq_d.ap()[blk])))
            for k in range(KC):
                late_dmas.append((k % 2, lambda k=k: (
                    nc.sync if k % 2 == 0 else nc.scalar).dma_start(
                        xT[:, k, TOKG : 2 * TOKG],
                        xT_d.ap()[:, k, TOKG : 2 * TOKG])))
            for blk in range(KC):
                late_dmas.append((blk % 2, lambda blk=blk: (
                    nc.sync if blk % 2 == 0 else nc.scalar).dma_start(
                        wp_sb[:, blk], wq_d if False else wp_d.ap()[blk])))
            for k in range(KC):
                late_dmas.append((k % 2, lambda k=k: (
                    nc.sync if k % 2 == 0 else nc.scalar).dma_start(
                        xT[:, k, 3 * TOKG : 4 * TOKG],
                        xT_d.ap()[:, k, 3 * TOKG : 4 * TOKG])))

            def dribble_dmas(n):
                for _ in range(n):
                    if late_dmas:
                        late_dmas.pop(0)[1]()

            def emit_S_pair(g, w, half, qkT):
                """S^T = K_h^T.T @ Q_h^T for head-banks (2*half, 2*half+1)
                into one two-bank psS tile so a single ACT exp covers both."""
                psS = psSp.tile([128, 2, 512], f32, tag="psS")
                for j in range(2):
                    hb = 2 * half + j
                    for m in range(4):
                        h = 4 * hb + m
                        nc.tensor.matmul(
                            psS[:, j, 128 * m : 128 * (m + 1)],
                            qkT[:, 16 + h, 128 * w : 128 * (w + 1)],
                            qkT[:, h, 128 * w : 128 * (w + 1)],
                            start=True,
                            stop=True,
                        )
                return psS

            def emit_proj(g, w):
                otile = o_pool.tile([128, C], bf, tag="o")
                for nk in range(2):
                    ps = psA.tile([128, 512], f32, tag="psA")
                    for k in range(KC):
                        nc.tensor.matmul(
                            ps[:],
                            owT[:, k, 128 * w : 128 * (w + 1)],
                            wp_sb[:, 4 * nk : 4 * (nk + 1), k, :],
                            start=(k == 0),
                            stop=(k == KC - 1 and not has_projb),
                        )
                    if has_projb:
                        nc.tensor.matmul(
                            ps[:],
                            onest[0:1, 0:128],
                            projb_sb[0:1, 512 * nk : 512 * (nk + 1)],
                            start=False,
                            stop=True,
                        )
                    with nc.allow_low_precision(reason="bf16 out"):
                        nc.vector.tensor_copy(
                            otile[:, 512 * nk : 512 * (nk + 1)], ps[:]
                        )
                nc.sync.dma_start(out_v[g, w], otile[:])

            def emit_recip(psVt, g, w, pair):
                """Normalize the two head-banks (2*pair, 2*pair+1) of window
                w: ACT ln + exp(-x) reciprocal of the den rows, K=1 bf16
                matmul partition broadcast, DVE copy to SBUF, DVE multiply
                into owT."""
                L = rL_pool.tile([1, 2, 512], f32, tag="rL")
                nc.scalar.activation(L[:], psVt[64:65, :, :], Ln)
                rall = r_pool.tile([1, 2, 512], bf, tag="r")
                with nc.allow_low_precision(reason="bf16 softmax recip"):
                    nc.scalar.activation(rall[:], L[:], Exp, scale=-1.0)
                Rall = R_pool.tile([64, 2, 512], bf, tag="R")
                for s_ in range(2):
                    Rp = psA.tile([64, 512], f32, tag="psA")
                    nc.tensor.matmul(
                        Rp[:],
                        ones_col[0:1, 0:64],
                        rall[:, s_, :],
                        start=True,
                        stop=True,
                    )
                    with nc.allow_low_precision(reason="bf16 recip row"):
                        nc.vector.tensor_copy(Rall[:, s_, :], Rp[:])
                for s in range(2):
                    hb = 2 * pair + s
                    psVv = psVt[:, s, :].rearrange(
                        "p (s2 two m) -> p s2 two m", two=2, m=128
                    )
                    Rv = Rall[:, s, :].rearrange(
                        "p (s2 two m) -> p s2 two m", two=2, m=128
                    )
                    for par in range(2):
                        with nc.allow_low_precision(reason="bf16 attn out"):
                            nc.vector.tensor_tensor(
                                owT[
                                    64 * par : 64 * (par + 1),
                                    2 * hb : 2 * hb + 2,
                                    128 * w : 128 * (w + 1),
                                ],
                                psVv[0:64, :, par, :],
                                Rv[:, :, par, :],
                                op=mybir.AluOpType.mult,
                            )

            projlog = []   # windows whose normalization has been emitted

            def drain_proj():
                while projlog:
                    g_, w_ = projlog.pop(0)
                    emit_proj(g_, w_)

            def qkv_items(g, qkT, v65):
                """Thunks for group g's QKV projection: 16 QK chunks + 4 V
                token-chunk units (both nk halves each)."""
                def qk_chunk(c):
                    def go():
                        ps = psA.tile([128, 512], f32, tag="psA")
                        for k in range(KC):
                            nc.tensor.matmul(
                                ps[:],
                                wqk_t[c][:, k, :],
                                xT[:, k, TOKG * g : TOKG * (g + 1)],
                                start=(k == 0),
                                stop=(k == KC - 1 and not has_qkvb),
                            )
                        if has_qkvb:
                            nc.tensor.matmul(
                                ps[:],
                                qkvb_sb[0:1, 128 * c : 128 * (c + 1)],
                                onest[0:1, :],
                                start=False,
                                stop=True,
                            )
                        with nc.allow_low_precision(reason="bf16 qk evict"):
                            nc.vector.tensor_copy(qkT[:, 2 * c, :], ps[0:64, :])
                            nc.scalar.copy(qkT[:, 2 * c + 1, :], ps[64:128, :])
                    return go

                def v_unit(tc_):
                    def go():
                        for nk in range(2):
                            ps = psA.tile([128, 512], f32, tag="psA")
                            for k in range(KC):
                                nc.tensor.matmul(
                                    ps[:],
                                    xT[
                                        :,
                                        k,
                                        TOKG * g + 128 * tc_ :
                                        TOKG * g + 128 * (tc_ + 1),
                                    ],
                                    wqv_sb[:, 4 * nk : 4 * (nk + 1), k, :],
                                    start=(k == 0),
                                    stop=(k == KC - 1 and not has_qkvb),
                                )
                            if has_qkvb:
                                nc.tensor.matmul(
                                    ps[:],
                                    onest[0:1, 0:128],
                                    qkvb_sb[
                                        0:1,
                                        2 * C + 512 * nk : 2 * C + 512 * (nk + 1),
                                    ],
                                    start=False,
                                    stop=True,
                                )
                            with nc.allow_low_precision(reason="bf16 v evict"):
                                nc.scalar.copy(
                                    v65[:, tc_, 8 * nk : 8 * (nk + 1), 0:HD],
                                    ps[:].rearrange("p (h e) -> p h e", e=HD),
                                )
                    return go

                return [qk_chunk(c) for c in range(16)] + \
                    [v_unit(t_) for t_ in range(GW)]

            # merged pipeline: iteration t computes QKV(t) interleaved with
            # the attention stream of group t-1
            banks = {}
            qkTs = {}
            v65s = {}
            for t in range(NGRP + 1):
                if t > 1:
                    # B-pair of the previous attn group's last window
                    emit_recip(psVB, t - 2, GW - 1, 1)
                    projlog.append((t - 2, GW - 1))
                items = []
                if t < NGRP:
                    qkT_t = qkt_pool.tile([64, 4 * KC, TOKG], bf, tag="qkT")
                    v65_t = v_pool.tile([128, GW, NH, HD + 1], bf, tag="v65")
                    qkTs[t] = qkT_t
                    v65s[t] = v65_t
                    nc.scalar.copy(
                        v65_t[:, :, :, HD : HD + 1],
                        ones_col[:].rearrange("p (g h) -> p g h", g=GW)[
                            :, :, :, None
                        ],
                    )
                    items = qkv_items(t, qkT_t, v65_t)
                if t == 0:
                    for it in items:
                        it()
                        dribble_dmas(3)
                    dribble_dmas(99)
                else:
                    g = t - 1
                    qkT_g = qkTs[g]
                    v65_g = v65s[g]
                    ii = 0
                    E2 = None
                    for i in range(16):
                        w, hb = divmod(i, 4)
                        if hb % 2 == 0:
                            psS = banks.pop((w, hb // 2))
                            E2 = e_pool.tile([128, 2, 512], bf, tag="E")
                            with nc.allow_low_precision(reason="bf16 attn wts"):
                                nc.scalar.activation(E2[:], psS[:], Exp)
                        if hb == 1:
                            banks[(w, 1)] = emit_S_pair(g, w, 1, qkT_g)
                        elif hb == 3 and w + 1 < GW:
                            banks[(w + 1, 0)] = emit_S_pair(g, w + 1, 0, qkT_g)
                        psVt = psVA if hb < 2 else psVB
                        for m in range(4):
                            h = 4 * hb + m
                            nc.tensor.matmul(
                                psVt[0:65, hb % 2, 128 * m : 128 * (m + 1)],
                                v65_g[:, w, h, :],
                                E2[:, hb % 2, 128 * m : 128 * (m + 1)],
                                start=True,
                                stop=True,
                            )
                        if hb == 0 and w > 0:
                            emit_recip(psVB, g, w - 1, 1)
                            projlog.append((g, w - 1))
                        elif hb == 3:
                            drain_proj()       # proj(w-1) and stragglers
                            emit_recip(psVA, g, w, 0)
                        # interleave 1-2 QKV items of group t after each unit
                        take = 2 if i % 4 == 3 else 1
                        for _ in range(take):
                            if ii < len(items):
                                items[ii]()
                                ii += 1
                    while ii < len(items):
                        items[ii]()
                        ii += 1
                if t < NGRP:
                    banks[(0, 0)] = emit_S_pair(t, 0, 0, qkTs[t])
            emit_recip(psVB, NGRP - 1, GW - 1, 1)
            projlog.append((NGRP - 1, GW - 1))
            drain_proj()

    _split_drain_waits(nc, mybir)
    return nc


def _get_nc(has_qkvb, has_projb):
    key = (has_qkvb, has_projb)
    if key not in _BUILD_CACHE:
        _BUILD_CACHE[key] = _build(has_qkvb, has_projb)
    return _BUILD_CACHE[key]


def _host_prep(x, qkv_w, qkv_b, proj_w, proj_b):
    """Pre-transpose / permute / cast everything the kernel needs."""
    import ml_dtypes

    bf = ml_dtypes.bfloat16
    qw = np.asarray(qkv_w, np.float32).copy()
    qw[0:C] *= SCALE                       # fold softmax scale into W_q
    # [3C, C] -> [C, 3C] -> [kc, p, 3C] -> oc blocks [24, p, kc, 128]
    wqT = np.ascontiguousarray(qw.T)
    wq_blk = np.ascontiguousarray(
        wqT.reshape(KC, 128, 3 * KC, 128).transpose(2, 1, 0, 3)
    ).astype(bf)
    wpT = np.ascontiguousarray(np.asarray(proj_w, np.float32).T)
    wp_blk = np.ascontiguousarray(
        wpT.reshape(KC, 128, KC, 128).transpose(2, 1, 0, 3)
    ).astype(bf)

    x = np.asarray(x, np.float32)
    # per-core window-permuted x^T: [128 p, kc, 2048 tok]
    xTs = []
    for core in range(NCORES):
        b, it = divmod(core, T // WT)
        slab = x[b, it * SLAB : (it + 1) * SLAB, :]
        # (tt, ih, hh, iw, ww, c) -> (ih, iw, tt, hh, ww, c)
        perm = slab.reshape(WT, 4, WH, 4, WW, C).transpose(1, 3, 0, 2, 4, 5)
        xt = perm.reshape(SLAB, C).T                     # [C, 2048]
        xt = xt.reshape(KC, 128, SLAB).transpose(1, 0, 2)  # [p, kc, tok]
        xTs.append(np.ascontiguousarray(xt).astype(bf))

    qb = np.asarray(qkv_b, np.float32).copy()
    qb[0:C] *= SCALE
    pb = np.asarray(proj_b, np.float32)
    return xTs, wq_blk, wp_blk, qb.astype(bf).reshape(1, 3 * C), \
        pb.astype(bf).reshape(1, C)


def _host_unpermute(rows):
    """[2048, C] window-major bf16 rows -> slab token order f32."""
    a = np.asarray(rows, np.float32)
    # rows are (ih, iw, tt, hh, ww); invert to (tt, ih, hh, iw, ww)
    a = a.reshape(4, 4, WT, WH, WW, C).transpose(2, 0, 3, 1, 4, 5)
    return np.ascontiguousarray(a.reshape(SLAB, C))


def kernel(x, qkv_w, qkv_b, proj_w, proj_b, t, h, w, **_unused):
    from concourse.bass_utils import run_bass_kernel_spmd

    x = np.asarray(x, dtype=np.float32)
    assert x.shape == (B, N, C), x.shape
    assert int(t) == T and int(h) == H and int(w) == W

    qkv_b = np.asarray(qkv_b, dtype=np.float32)
    proj_b = np.asarray(proj_b, dtype=np.float32)
    has_qkvb = bool(np.any(qkv_b))
    has_projb = bool(np.any(proj_b))
    nc = _get_nc(has_qkvb, has_projb)

    xTs, wq_blk, wp_blk, qb, pb = _host_prep(x, qkv_w, qkv_b, proj_w, proj_b)

    in_maps = []
    for core in range(NCORES):
        im = {"xT": xTs[core], "wqkvT": wq_blk, "projT": wp_blk}
        if has_qkvb:
            im["qkvb"] = qb
        if has_projb:
            im["projb"] = pb
        in_maps.append(im)

    res = run_bass_kernel_spmd(nc, in_maps, core_ids=list(range(NCORES)))

    y = np.empty((B, N, C), dtype=np.float32)
    for core in range(NCORES):
        b, it = divmod(core, T // WT)
        y[b, it * SLAB : (it + 1) * SLAB, :] = _host_unpermute(
            res.results[core]["out"]
        )
    return y
